# revision 31
# baseline (speedup 1.0000x reference)
"""GAT (nn_GAT_29523605193094) Trainium2 kernel.

The reference keeps the source bug ``src, dst = edges[0], edges[0]``, so the
adjacency matrix is purely diagonal: adj[i, i] = (i appears in edges[0]).
After the -inf masking, row i of the [N, N, H] score tensor has exactly one
finite entry (j = i) when node i is covered, so softmax over axis=1 yields
exactly 1.0 at (i, i), and the output row is exactly h[i] = (X @ W)[i].
Rows for uncovered nodes are all -inf -> softmax is NaN -> NaN output row.
Both cases are reproduced here:

    out = X @ W            (on 8 NeuronCores, row-sharded, bf16 matmul)
    out[~covered] = NaN    (host-side mask from edges[0])

Device-side structure (per core: [512, 512] @ [512, 256] in bf16, fp32
PSUM accumulate, bf16 output, host casts), tuned against NTFF traces:

- Inputs are host-packed partition-major and k-interleaved: one DRAM
  block per k-chunk carrying [w_k | xt_k] as a single 1.5 KiB-line
  transfer, so each k-sweep of the matmul loop gates on exactly ONE
  DMA completion semaphore (each costs ~900 ns past the data).
- The 4 block transfers ride 3 HWDGE queues so their ~600 ns configs
  and completions overlap, ordered so chunk k lands just ahead of the
  PE's k-sweep.
- The Tensor engine p-state ramps (0.65 -> 1.2 -> 2.4 GHz after ~3-5 us
  of continuous work) and ANY idle gap resets the ramp, so a chain of
  scratch-fed warmup matmuls runs from body start until the first
  operands land; the real matmuls then run at/near full clock.
  Warmup targets the real PSUM banks (safe: the real k0 matmul uses
  start=True, which resets the bank).
- PSUM->SBUF bf16 downcasts run on vector only: the scalar (ACT)
  engine would pull a 1.28 us ACT table-load DMA into the body start,
  contending with the input stream (measured ~0.5 us net loss).
- Per-m out DMAs spread over gpsimd/scalar/sync so the ~600 ns configs
  and ~900 ns completion-semaphore lags overlap; gpsimd (SWDGE, ~0.5 us
  slower completion) carries the earliest block.
"""

import numpy as np

N = 4096
IN = 512
OUT = 256
NCORES = 8
RB = N // NCORES  # 512 rows per core
P = 128
KT = IN // P      # 4 contraction chunks
MT = RB // P      # 4 output row blocks per core
BLK = OUT + RB    # one k-chunk block: [w_k | xt_k] per partition
WARMUP = 16       # bridges body start -> first operand arrival (~3.4 us)

_state = {}

# test.py reads this after a traced call for the HW exec time.
LAST_RESULTS = None


def _build():
    import concourse.mybir as mybir
    import concourse.tile as tile
    from concourse import bacc

    nc = bacc.Bacc(
        "TRN2",
        target_bir_lowering=False,
        debug=False,
        num_devices=NCORES,
    )
    f32 = mybir.dt.float32
    bf16 = mybir.dt.bfloat16

    # Bass unconditionally emits four const-AP memsets (0.0/1.0/bf16-1.0/
    # uint8-127) in the preamble; nothing in this kernel reads them (the
    # BIR verifier flags them as reader-less) and they sit on the gpsimd
    # critical path just before the body branch (~0.3-0.4 us). Drop them.
    ent = nc.m.functions[0].blocks[0]

    def _dead_preamble(ins):
        ty = type(ins).__name__
        if ty == "InstMemset" and getattr(ins.outs[0], "memref", "").startswith(
            "const-"
        ):
            return True
        # The trailing all-engine barrier of Bass.__init__ (drain +
        # rendezvous per engine): with the const memsets gone there is no
        # cross-engine preamble state left to fence — the tile body's own
        # semaphores order everything that follows.
        name = getattr(ins, "name", "")
        return ty == "InstDrain" or (
            ty == "InstEventSemaphore" and name.startswith("barrier_")
        )

    kept = [ins for ins in ent.instructions if not _dead_preamble(ins)]
    del ent.instructions[:]
    for ins in kept:
        ent.instructions.append(ins)
    # Partition-major packing: row p of block k holds K-row k*128+p of
    # [W | X_shard^T] (resp. M-row m*128+p of out for the output).
    inp = nc.dram_tensor("inp", [P, KT * BLK], bf16, kind="ExternalInput")
    out = nc.dram_tensor("out", [P, MT * OUT], bf16, kind="ExternalOutput")

    with tile.TileContext(nc) as tc:
        with (
            tc.tile_pool(name="ins", bufs=1) as in_pool,
            tc.tile_pool(name="outs", bufs=1) as out_pool,
            tc.tile_pool(name="ps", bufs=4, space="PSUM") as psum_pool,
        ):
            inp_t = in_pool.tile([P, KT, BLK], bf16)
            scratch = in_pool.tile([P, OUT], bf16)
            # Scratch memset on vector (idle until the output phase): a
            # gpsimd memset would delay the k2 block's DMA config behind it.
            nc.vector.memset(scratch[:], 0)

            in_q = [nc.sync, nc.scalar, nc.gpsimd, nc.sync]
            for k in range(KT):
                in_q[k].dma_start(
                    inp_t[:, k, :], inp[:, k * BLK : (k + 1) * BLK]
                )

            pss = [
                psum_pool.tile([P, OUT], f32, name=f"ps{m}", tag="ps")
                for m in range(MT)
            ]
            # P-state warmup: keep the PE continuously busy from body start
            # so it reaches full clock by the time real operands land.
            for i in range(WARMUP):
                nc.tensor.matmul(
                    pss[i % MT][:],
                    scratch[:, 0:P],
                    scratch[:],
                    start=True,
                    stop=True,
                )

            # k-outer / m-inner: when the last k chunk lands only one
            # m-sweep (4 matmuls) remains, minimizing the post-DMA tail.
            for k in range(KT):
                for m in range(MT):
                    nc.tensor.matmul(
                        pss[m][:],
                        inp_t[:, k, OUT + m * P : OUT + (m + 1) * P],
                        inp_t[:, k, 0:OUT],
                        start=(k == 0),
                        stop=(k == KT - 1),
                    )
            ob = out_pool.tile([P, MT, OUT], bf16)
            # Per-m out DMAs: gpsimd's (SWDGE) completion path is ~0.5 us
            # slower than the HWDGE queues, so it carries the EARLIEST
            # block; the rest spread over scalar/sync so configs and
            # ~900 ns completion-sem lags overlap.
            out_q = [nc.gpsimd, nc.scalar, nc.sync, nc.scalar]
            for m in range(MT):
                # All PSUM->SBUF downcasts on vector: using the scalar ACT
                # engine would pull a 1.28 us ACT table-load DMA into the
                # body start, contending with the input stream.
                nc.vector.tensor_copy(ob[:, m, :], pss[m][:])
                out_q[m].dma_start(
                    out[:, m * OUT : (m + 1) * OUT], ob[:, m, :]
                )

    nc.compile()
    return nc


def kernel(X, edges, W, A):
    global LAST_RESULTS
    import ml_dtypes
    from concourse.bass_utils import run_bass_kernel_spmd

    X = np.asarray(X, dtype=np.float32)
    W = np.asarray(W, dtype=np.float32)
    edges = np.asarray(edges)

    if "nc" not in _state:
        _state["nc"] = _build()
    nc = _state["nc"]

    bf = ml_dtypes.bfloat16
    XT = X.T.astype(bf)                       # [IN, N]
    Wb = W.astype(bf)                         # [IN, OUT]
    # Per-core block packing: inp[p, k*BLK : (k+1)*BLK] = [W | X_c^T] row
    # k*128+p, i.e. [KT*P, OUT+RB] -> [P, KT*BLK] partition-major.
    in_maps = []
    for c in range(NCORES):
        cat = np.concatenate([Wb, XT[:, c * RB : (c + 1) * RB]], axis=1)
        in_maps.append(
            {
                "inp": np.ascontiguousarray(
                    cat.reshape(KT, P, BLK).transpose(1, 0, 2).reshape(P, KT * BLK)
                )
            }
        )
    # The device occasionally reports a transient NRT_EXEC_UNIT_UNRECOVERABLE
    # on an otherwise-good kernel; retry before giving up.
    last_exc = None
    for _attempt in range(3):
        try:
            res = run_bass_kernel_spmd(nc, in_maps, core_ids=list(range(NCORES)))
            break
        except Exception as exc:  # noqa: BLE001
            last_exc = exc
            import time

            time.sleep(2.0)
    else:
        raise last_exc
    LAST_RESULTS = res
    out = np.concatenate(
        [
            res.results[c]["out"]
            .astype(np.float32)
            .reshape(P, MT, OUT)
            .transpose(1, 0, 2)
            .reshape(RB, OUT)
            for c in range(NCORES)
        ],
        axis=0,
    )

    # Reference semantics: nodes absent from edges[0] have an all -inf score
    # row; softmax of that is NaN, which propagates to the output row.
    covered = np.zeros(N, dtype=bool)
    covered[edges[0]] = True
    if not covered.all():
        out[~covered] = np.nan
    return out


# revision 32
# speedup vs baseline: 1.0136x; 1.0136x over previous
"""GAT (nn_GAT_29523605193094) Trainium2 kernel.

The reference keeps the source bug ``src, dst = edges[0], edges[0]``, so the
adjacency matrix is purely diagonal: adj[i, i] = (i appears in edges[0]).
After the -inf masking, row i of the [N, N, H] score tensor has exactly one
finite entry (j = i) when node i is covered, so softmax over axis=1 yields
exactly 1.0 at (i, i), and the output row is exactly h[i] = (X @ W)[i].
Rows for uncovered nodes are all -inf -> softmax is NaN -> NaN output row.
Both cases are reproduced here:

    out = X @ W            (on 8 NeuronCores, row-sharded, bf16 matmul)
    out[~covered] = NaN    (host-side mask from edges[0])

Device-side structure (per core: [512, 512] @ [512, 256] in bf16, fp32
PSUM accumulate, bf16 output, host casts), tuned against NTFF traces:

- Inputs are host-packed partition-major and k-interleaved: one DRAM
  block per k-chunk carrying [w_k | xt_k] as a single 1.5 KiB-line
  transfer, so each k-sweep of the matmul loop gates on exactly ONE
  DMA completion semaphore (each costs ~900 ns past the data).
- The 4 block transfers ride 3 HWDGE queues so their ~600 ns configs
  and completions overlap, ordered so chunk k lands just ahead of the
  PE's k-sweep.
- The Tensor engine p-state ramps (0.65 -> 1.2 -> 2.4 GHz after ~3-5 us
  of continuous work) and ANY idle gap resets the ramp, so a chain of
  scratch-fed warmup matmuls runs from body start until the first
  operands land; the real matmuls then run at/near full clock.
  Warmup targets the real PSUM banks (safe: the real k0 matmul uses
  start=True, which resets the bank).
- PSUM->SBUF bf16 downcasts run on vector only: the scalar (ACT)
  engine would pull a 1.28 us ACT table-load DMA into the body start,
  contending with the input stream (measured ~0.5 us net loss).
- Per-m out DMAs spread over gpsimd/scalar/sync so the ~600 ns configs
  and ~900 ns completion-semaphore lags overlap; gpsimd (SWDGE, ~0.5 us
  slower completion) carries the earliest block.
"""

import numpy as np

N = 4096
IN = 512
OUT = 256
NCORES = 8
RB = N // NCORES  # 512 rows per core
P = 128
KT = IN // P      # 4 contraction chunks
MT = RB // P      # 4 output row blocks per core
BLK = OUT + RB    # one k-chunk block: [w_k | xt_k] per partition
WARMUP = 16       # bridges body start -> first operand arrival (~3.4 us)

_state = {}

# test.py reads this after a traced call for the HW exec time.
LAST_RESULTS = None


def _build():
    import concourse.mybir as mybir
    import concourse.tile as tile
    from concourse import bacc

    nc = bacc.Bacc(
        "TRN2",
        target_bir_lowering=False,
        debug=False,
        num_devices=NCORES,
    )
    f32 = mybir.dt.float32
    bf16 = mybir.dt.bfloat16

    # Bass unconditionally emits four const-AP memsets (0.0/1.0/bf16-1.0/
    # uint8-127) in the preamble; nothing in this kernel reads them (the
    # BIR verifier flags them as reader-less) and they sit on the gpsimd
    # critical path just before the body branch (~0.3-0.4 us). Drop them.
    ent = nc.m.functions[0].blocks[0]
    kept = [
        ins
        for ins in ent.instructions
        if not (
            type(ins).__name__ == "InstMemset"
            and getattr(ins.outs[0], "memref", "").startswith("const-")
        )
    ]
    del ent.instructions[:]
    for ins in kept:
        ent.instructions.append(ins)
    # Partition-major packing: row p of block k holds K-row k*128+p of
    # [W | X_shard^T] (resp. M-row m*128+p of out for the output).
    inp = nc.dram_tensor("inp", [P, KT * BLK], bf16, kind="ExternalInput")
    out = nc.dram_tensor("out", [P, MT * OUT], bf16, kind="ExternalOutput")

    with tile.TileContext(nc) as tc:
        with (
            tc.tile_pool(name="ins", bufs=1) as in_pool,
            tc.tile_pool(name="outs", bufs=1) as out_pool,
            tc.tile_pool(name="ps", bufs=4, space="PSUM") as psum_pool,
        ):
            inp_t = in_pool.tile([P, KT, BLK], bf16)
            scratch = in_pool.tile([P, OUT], bf16)
            # Scratch memset on vector (idle until the output phase): a
            # gpsimd memset would delay the k2 block's DMA config behind it.
            nc.vector.memset(scratch[:], 0)

            in_q = [nc.sync, nc.scalar, nc.gpsimd, nc.sync]
            for k in range(KT):
                in_q[k].dma_start(
                    inp_t[:, k, :], inp[:, k * BLK : (k + 1) * BLK]
                )

            pss = [
                psum_pool.tile([P, OUT], f32, name=f"ps{m}", tag="ps")
                for m in range(MT)
            ]
            # P-state warmup: keep the PE continuously busy from body start
            # so it reaches full clock by the time real operands land.
            for i in range(WARMUP):
                nc.tensor.matmul(
                    pss[i % MT][:],
                    scratch[:, 0:P],
                    scratch[:],
                    start=True,
                    stop=True,
                )

            # k-outer / m-inner: when the last k chunk lands only one
            # m-sweep (4 matmuls) remains, minimizing the post-DMA tail.
            for k in range(KT):
                for m in range(MT):
                    nc.tensor.matmul(
                        pss[m][:],
                        inp_t[:, k, OUT + m * P : OUT + (m + 1) * P],
                        inp_t[:, k, 0:OUT],
                        start=(k == 0),
                        stop=(k == KT - 1),
                    )
            ob = out_pool.tile([P, MT, OUT], bf16)
            # Per-m out DMAs: gpsimd's (SWDGE) completion path is ~0.5 us
            # slower than the HWDGE queues, so it carries the EARLIEST
            # block; the rest spread over scalar/sync so configs and
            # ~900 ns completion-sem lags overlap.
            out_q = [nc.gpsimd, nc.scalar, nc.sync, nc.scalar]
            for m in range(MT):
                # All PSUM->SBUF downcasts on vector: using the scalar ACT
                # engine would pull a 1.28 us ACT table-load DMA into the
                # body start, contending with the input stream.
                nc.vector.tensor_copy(ob[:, m, :], pss[m][:])
                out_q[m].dma_start(
                    out[:, m * OUT : (m + 1) * OUT], ob[:, m, :]
                )

    nc.compile()
    return nc


def kernel(X, edges, W, A):
    global LAST_RESULTS
    import ml_dtypes
    from concourse.bass_utils import run_bass_kernel_spmd

    X = np.asarray(X, dtype=np.float32)
    W = np.asarray(W, dtype=np.float32)
    edges = np.asarray(edges)

    if "nc" not in _state:
        _state["nc"] = _build()
    nc = _state["nc"]

    bf = ml_dtypes.bfloat16
    XT = X.T.astype(bf)                       # [IN, N]
    Wb = W.astype(bf)                         # [IN, OUT]
    # Per-core block packing: inp[p, k*BLK : (k+1)*BLK] = [W | X_c^T] row
    # k*128+p, i.e. [KT*P, OUT+RB] -> [P, KT*BLK] partition-major.
    in_maps = []
    for c in range(NCORES):
        cat = np.concatenate([Wb, XT[:, c * RB : (c + 1) * RB]], axis=1)
        in_maps.append(
            {
                "inp": np.ascontiguousarray(
                    cat.reshape(KT, P, BLK).transpose(1, 0, 2).reshape(P, KT * BLK)
                )
            }
        )
    # The device occasionally reports a transient NRT_EXEC_UNIT_UNRECOVERABLE
    # on an otherwise-good kernel; retry before giving up.
    last_exc = None
    for _attempt in range(3):
        try:
            res = run_bass_kernel_spmd(nc, in_maps, core_ids=list(range(NCORES)))
            break
        except Exception as exc:  # noqa: BLE001
            last_exc = exc
            import time

            time.sleep(2.0)
    else:
        raise last_exc
    LAST_RESULTS = res
    out = np.concatenate(
        [
            res.results[c]["out"]
            .astype(np.float32)
            .reshape(P, MT, OUT)
            .transpose(1, 0, 2)
            .reshape(RB, OUT)
            for c in range(NCORES)
        ],
        axis=0,
    )

    # Reference semantics: nodes absent from edges[0] have an all -inf score
    # row; softmax of that is NaN, which propagates to the output row.
    covered = np.zeros(N, dtype=bool)
    covered[edges[0]] = True
    if not covered.all():
        out[~covered] = np.nan
    return out
